# revision 19
# baseline (speedup 1.0000x reference)
"""Trainium2 Bass kernel for nn_AttentionResidualBlock (B=16, C=256, H=W=32, heads=8).

Sharding: data-parallel over batch across 8 NeuronCores (2 images/core),
weights replicated; attention heads processed in pairs on-chip.

Per core:
  - conv3x3 as 9 shifted bf16 matmuls over a zero-padded [C, 34, 34] layout;
    BN scale folded into weights on host, BN shift + ReLU fused on DVE.
    Conv work is interleaved into the attention pairs as TensorE filler.
  - attention: scoresT[m,n] = k^T q per head; the two heads of a pair are
    issued back-to-back on different PE row-groups (K=32 row tiling) into
    one PSUM tile; exp on ScalarE (PSUM -> SBUF bf16); attention output +
    softmax row-sum in one accumulation via a ones-augmented v (M=33,
    col positions 0/64 for the pair); denominators applied via DVE
    reciprocal + broadcast DMA (through a DRAM scratch) + multiply.
  - out-projection consumes the on-chip head layout directly; out_w is
    row-permuted (and gate/out-bias/v-bias folded) on host.
All matmuls bf16 with fp32 PSUM accumulation (~2.2e-3 scale-rel err vs fp32).
"""

import numpy as np
import ml_dtypes
from contextlib import ExitStack

import concourse.bass as bass
import concourse.bacc as bacc
import concourse.mybir as mybir
import concourse.tile as tile
from concourse.bass_utils import run_bass_kernel_spmd

F32 = mybir.dt.float32
BF16 = mybir.dt.bfloat16
AF = mybir.ActivationFunctionType
ALU = mybir.AluOpType

C = 256
HEADS = 8
D = 32
B, H, W = 16, 32, 32
N = H * W          # 1024
HP = H + 2         # 34
EPS = 1e-5
N_CORES = 8
IMGS = B // N_CORES  # 2 images per core
CC = C // 128      # 2 channel chunks
MC = N // 128      # 8 spatial m-chunks
DAUG = D + 1       # 33 (v rows + ones row)

# packed bf16 weight layout (columns per partition)
W1_COLS = CC * 9 * CC * 128          # 4608
QKVO_COLS = CC * C                   # 512
PACK_COLS = 2 * W1_COLS + 3 * QKVO_COLS + 4 * C  # w1 w2 q k v ow


def _bcast_ap(sliced: bass.AP, parts: int) -> bass.AP:
    """Broadcast a DRAM [F] AP to [parts, F] (prepended step-0 dim)."""
    ap = [[0, parts]] + [list(d) for d in sliced.ap]
    return bass.AP(tensor=sliced.tensor, offset=sliced.offset, ap=ap)


def build_nc() -> bass.Bass:
    nc = bacc.Bacc()

    x_d = nc.declare_dram_parameter("x_sh", [IMGS, CC, 128, N], F32, isOutput=False)
    wp_d = nc.declare_dram_parameter("wpack", [128, PACK_COLS], BF16, isOutput=False)
    vec_d = nc.declare_dram_parameter("vecs", [128, 3 * CC], F32, isOutput=False)
    out_d = nc.declare_dram_parameter("out_sh", [IMGS, CC, 128, N], F32, isOutput=True)

    with ExitStack() as ctx:
        tc = ctx.enter_context(tile.TileContext(nc))
        wpool = ctx.enter_context(tc.tile_pool(name="weights", bufs=1))
        xpool = ctx.enter_context(tc.tile_pool(name="acts", bufs=2))
        ptpool = ctx.enter_context(tc.tile_pool(name="pt", bufs=1))
        ps_sc = ctx.enter_context(tc.tile_pool(name="ps_sc", bufs=2, space="PSUM"))
        ps_at = ctx.enter_context(tc.tile_pool(name="ps_at", bufs=1, space="PSUM"))
        ps_cv = ctx.enter_context(tc.tile_pool(name="ps_cv", bufs=1, space="PSUM"))
        dpool = ctx.enter_context(tc.tile_pool(name="dram", bufs=2, space="DRAM"))

        # ---- x loads first (the first compute consumer), then weights ----
        xtiles = {}

        def xload(img):
            xpad = xpool.tile([128, CC, HP, HP], F32, tag="xpad", name="xpad")
            xpadb = xpool.tile([128, CC, HP, HP], BF16, tag="xpadb", name="xpadb")
            for cc in range(CC):
                nc.vector.memset(xpad[:, cc], 0.0)
                nc.sync.dma_start(
                    out=xpad[:, cc, 1:HP - 1, 1:HP - 1],
                    in_=x_d[img, cc].rearrange("p (r c) -> p r c", r=H))
                nc.vector.tensor_copy(xpadb[:, cc], xpad[:, cc])
            xtiles[img] = (xpad, xpadb)

        xload(0)
        wpack = wpool.tile([128, PACK_COLS], BF16, tag="wpack")
        nc.sync.dma_start(out=wpack[:, 2 * W1_COLS:], in_=wp_d[:, 2 * W1_COLS:])
        nc.sync.dma_start(out=wpack[:, :2 * W1_COLS], in_=wp_d[:, :2 * W1_COLS])
        o_w1, o_w2 = 0, W1_COLS
        o_q = 2 * W1_COLS
        o_k, o_v = o_q + QKVO_COLS, o_q + 2 * QKVO_COLS
        o_ow = o_q + 3 * QKVO_COLS

        def conv_w(base, ic, tap, oc):  # [128, 128] lhsT slice
            off = base + ((ic * 9 + tap) * CC + oc) * 128
            return wpack[:, off:off + 128]

        vecs = wpool.tile([128, 3 * CC], F32, tag="vecs")
        nc.sync.dma_start(out=vecs, in_=vec_d[:])
        shift1 = lambda oc: vecs[:, oc:oc + 1]
        shiftF = lambda oc: vecs[:, CC + oc:CC + oc + 1]
        qbias = lambda oc: vecs[:, 2 * CC + oc:2 * CC + oc + 1]
        rs_tmp = wpool.tile([128, N], F32, tag="rstmp")
        bcast = wpool.tile([128, N], F32, tag="bcast")
        cmb1 = wpool.tile([128, N], F32, tag="cmb1")

        xload(1)

        for img in range(IMGS):
            xpad, xpadb = xtiles[img]

            def xflat(t, cc):  # unpadded [p, 32, 32] view
                return t[:, cc, 1:HP - 1, 1:HP - 1]

            # ---- q, k ----
            q_sb = xpool.tile([128, CC, N], BF16, tag="q")
            k_sb = xpool.tile([128, CC, N], BF16, tag="k")
            for oc in range(CC):
                for dst, wb, bias in ((q_sb, o_q, qbias(oc)), (k_sb, o_k, None)):
                    ps = ps_sc.tile([128, N], F32, tag="sc", name="psqk")
                    for nh in range(2):
                        for ic in range(CC):
                            nc.tensor.matmul(
                                ps[:, nh * 512:(nh + 1) * 512],
                                lhsT=wpack[:, wb + ic * C + oc * 128:
                                           wb + ic * C + (oc + 1) * 128],
                                rhs=xflat(xpadb, ic)[:, nh * 16:(nh + 1) * 16, :],
                                start=(ic == 0), stop=(ic == CC - 1))
                    if bias is not None:
                        nc.vector.tensor_scalar(dst[:, oc], ps, bias, None, ALU.add)
                    else:
                        nc.vector.tensor_copy(dst[:, oc], ps)

            # ---- vT (flat bf16 x for the stationary; ones col per head) ----
            xnb = xpool.tile([128, CC, N], BF16, tag="xnb")
            for cc in range(CC):
                nc.vector.tensor_copy(
                    xnb[:, cc].rearrange("p (r c) -> p r c", r=H), xflat(xpadb, cc))
            v_aug = xpool.tile([128, MC, HEADS, DAUG], BF16, tag="vaug")
            for mc in range(MC):
                nc.vector.memset(v_aug[:, mc], 1.0)
                ps = ps_sc.tile([128, C], F32, tag="sc", name="psv")
                for ic in range(CC):
                    nc.tensor.matmul(
                        ps,
                        lhsT=xnb[:, ic, mc * 128:(mc + 1) * 128],
                        rhs=wpack[:, o_v + ic * C: o_v + (ic + 1) * C],
                        start=(ic == 0), stop=(ic == CC - 1))
                nc.vector.tensor_copy(
                    v_aug[:, mc, :, 0:D],
                    ps.rearrange("p (h e) -> p h e", h=HEADS))

            # ---- conv pieces (PE filler inside attention pairs) ----
            c1pad = xpool.tile([128, CC, HP, HP], BF16, tag="c1pad")
            for cc in range(CC):
                nc.vector.memset(c1pad[:, cc], 0.0)
            c2x = xpool.tile([128, CC, N], F32, tag="c2x")  # conv2 + shiftF + x

            conv_ps = {}

            def conv_half(key, w_base, src_pad, oc, nh):
                if key not in conv_ps:
                    conv_ps[key] = ps_cv.tile([128, N], F32, tag="cv", name=key)
                ps = conv_ps[key]
                first = True
                for ic in range(CC):
                    for tap in range(9):
                        ky, kx = divmod(tap, 3)
                        nc.tensor.matmul(
                            ps[:, nh * 512:(nh + 1) * 512],
                            lhsT=conv_w(w_base, ic, tap, oc),
                            rhs=src_pad[:, ic, ky + nh * 16:ky + nh * 16 + 16, kx:kx + W],
                            start=first, stop=(ic == CC - 1 and tap == 8))
                        first = False
                return ps

            def c1_work(oc, nh):
                ps = conv_half(f"c1{oc}", o_w1, xpadb, oc, nh)
                nc.vector.tensor_scalar(
                    xflat(c1pad, oc)[:, nh * 16:(nh + 1) * 16, :],
                    ps.rearrange("p (r c) -> p r c", r=H)[:, nh * 16:(nh + 1) * 16, :],
                    shift1(oc), 0.0, ALU.add, ALU.max)
                if nh == 1:
                    del conv_ps[f"c1{oc}"]

            def c2_work(oc, nh):
                ps = conv_half(f"c2{oc}", o_w2, c1pad, oc, nh)
                nc.vector.scalar_tensor_tensor(
                    out=c2x[:, oc].rearrange("p (r c) -> p r c", r=H)[:, nh * 16:(nh + 1) * 16, :],
                    in0=ps.rearrange("p (r c) -> p r c", r=H)[:, nh * 16:(nh + 1) * 16, :],
                    scalar=shiftF(oc),
                    in1=xflat(xpad, oc)[:, nh * 16:(nh + 1) * 16, :],
                    op0=ALU.add, op1=ALU.add)
                if nh == 1:
                    del conv_ps[f"c2{oc}"]

            fillers = [
                [lambda: c1_work(0, 0), lambda: c1_work(0, 1)],
                [lambda: c1_work(1, 0), lambda: c1_work(1, 1)],
                [lambda: c2_work(0, 0), lambda: c2_work(0, 1)],
                [lambda: c2_work(1, 0), lambda: c2_work(1, 1)],
            ]

            # ---- attention: head pairs ----
            attn_t = [xpool.tile([128, N], BF16, tag=f"attn{p_}", name=f"attn{p_}")
                      for p_ in range(4)]
            for p_ in range(4):
                nc.vector.memset(attn_t[p_], 0.0)
            rs_dr = dpool.tile([4, 2, N], F32, tag="rsdram", name="rs_dr")

            for p_ in range(4):
                ha, hb = 2 * p_, 2 * p_ + 1
                hpa, cca = 32 * (ha % 4), ha // 4
                hpb, ccb = 32 * (hb % 4), hb // 4
                pt = {}
                for mc in range(MC):
                    for nh in range(2):
                        # one PSUM tile holds this n-half for both heads; the
                        # two K=32 matmuls sit on different PE row groups
                        sc = ps_sc.tile([128, N], F32, tag="sc", name="sc")
                        nsl = slice(nh * 512, (nh + 1) * 512)
                        nc.tensor.matmul(
                            sc[:, 0:512],
                            lhsT=k_sb[hpa:hpa + 32, cca, mc * 128:(mc + 1) * 128],
                            rhs=q_sb[hpa:hpa + 32, cca, nsl],
                            start=True, stop=True, tile_position=(hpa, 0))
                        nc.tensor.matmul(
                            sc[:, 512:1024],
                            lhsT=k_sb[hpb:hpb + 32, ccb, mc * 128:(mc + 1) * 128],
                            rhs=q_sb[hpb:hpb + 32, ccb, nsl],
                            start=True, stop=True, tile_position=(hpb, 0))
                        p = ptpool.tile([128, N], BF16, tag=f"pt{mc}_{nh}",
                                        name=f"pt{mc}_{nh}")
                        nc.scalar.activation(p, sc, AF.Exp)
                        pt[mc, nh] = p
                    if mc == 3:
                        fillers[p_].pop(0)()
                fillers[p_].pop(0)()

                # attn + rowsum into one [128, N] tile: head a rows 0:33,
                # head b rows 64:97 (sequential accumulation groups: a head's
                # start=True clears has_written bank-wide, so the other
                # head's group must not be mid-accumulation)
                at = ps_at.tile([128, N], F32, tag="at", name="at")
                for mc in range(MC):
                    for nh in range(2):
                        nsl = slice(nh * 512, (nh + 1) * 512)
                        for i, h in ((0, ha), (1, hb)):
                            nc.tensor.matmul(
                                at[64 * i:64 * i + DAUG, nsl],
                                lhsT=v_aug[:, mc, h, :],
                                rhs=pt[mc, nh][:, 512 * i:512 * i + 512],
                                start=(mc == 0), stop=(mc == MC - 1))

                # normalize: 1/rowsum (rows 32 / 96), broadcast, multiply
                for i in range(2):
                    r = 64 * i + D
                    nc.vector.reciprocal(rs_tmp[r:r + 1], at[r:r + 1])
                for i in range(2):
                    r = 64 * i + D
                    nc.sync.dma_start(out=rs_dr[p_, i], in_=rs_tmp[r:r + 1])
                    nc.sync.dma_start(
                        out=bcast[64 * i:64 * i + D],
                        in_=_bcast_ap(rs_dr[p_, i], D))
                for i in range(2):
                    nc.vector.tensor_tensor(
                        attn_t[p_][64 * i:64 * i + D],
                        at[64 * i:64 * i + D],
                        bcast[64 * i:64 * i + D],
                        ALU.mult)

            # ---- proj + combine ----
            out_sb = xpool.tile([128, CC, N], F32, tag="out")
            pj = {0: ps_cv.tile([128, N], F32, tag="cv", name="pj0"),
                  1: ps_sc.tile([128, N], F32, tag="sc", name="pj1")}
            def pj_mm(oc, nh, kc):
                nc.tensor.matmul(
                    pj[oc][:, nh * 512:(nh + 1) * 512],
                    lhsT=wpack[:, o_ow + kc * C + oc * 128:
                               o_ow + kc * C + (oc + 1) * 128],
                    rhs=attn_t[kc][:, nh * 512:(nh + 1) * 512],
                    start=(kc == 0), stop=(kc == 3))

            # kc 0..2 only need the first three pairs' attention output, so
            # they overlap the last pair's normalize; kc3 closes each group
            for oc in range(CC):
                for nh in range(2):
                    for kc in range(3):
                        pj_mm(oc, nh, kc)
            for oc in range(CC):
                for nh in range(2):
                    pj_mm(oc, nh, 3)
                nc.vector.tensor_tensor(cmb1, c2x[:, oc], pj[oc], ALU.add)
                nc.vector.tensor_scalar(out_sb[:, oc], cmb1, 0.0, None, ALU.max)
                nc.sync.dma_start(out=out_d[img, oc], in_=out_sb[:, oc])

    nc.finalize()
    return nc


def _prep_inputs(inputs: dict) -> list[dict]:
    bf = ml_dtypes.bfloat16
    x = np.asarray(inputs["x"], dtype=np.float32)
    f32 = lambda k: np.asarray(inputs[k], dtype=np.float32)
    bn1_inv = f32("bn1_gamma") / np.sqrt(f32("bn1_var") + EPS)
    shift1 = f32("bn1_beta") - f32("bn1_mean") * bn1_inv + f32("conv1_b") * bn1_inv
    w1s = f32("conv1_w") * bn1_inv[:, None, None, None]
    bn2_inv = f32("bn2_gamma") / np.sqrt(f32("bn2_var") + EPS)
    shift2 = f32("bn2_beta") - f32("bn2_mean") * bn2_inv + f32("conv2_b") * bn2_inv
    w2s = f32("conv2_w") * bn2_inv[:, None, None, None]
    sg = 1.0 / (1.0 + np.exp(-float(np.asarray(inputs["gate"]))))
    ow = f32("out_w") * sg
    shiftF = shift2 + sg * f32("out_b") + sg * (f32("out_w") @ f32("v_b"))
    qws = f32("q_w") / np.sqrt(D)
    qbs = f32("q_b") / np.sqrt(D)

    def conv_pack(w):  # [O, I, 3, 3] -> [128, CC*9*CC*128]
        t = w.transpose(1, 2, 3, 0).reshape(CC, 128, 3, 3, CC, 128)
        return t.transpose(1, 0, 2, 3, 4, 5).reshape(128, W1_COLS)

    def pack_T(w):  # [O, C_in] -> [128, CC*C]
        return w.T.reshape(CC, 128, C).transpose(1, 0, 2).reshape(128, QKVO_COLS)

    owT = ow.T  # [C_in, C_out]
    owp = np.zeros((4, 128, C), np.float32)
    for p_ in range(4):
        owp[p_, 0:32] = owT[64 * p_: 64 * p_ + 32]
        owp[p_, 64:96] = owT[64 * p_ + 32: 64 * p_ + 64]
    owpk = owp.transpose(1, 0, 2).reshape(128, 4 * C)

    wpack = np.concatenate(
        [conv_pack(w1s), conv_pack(w2s), pack_T(qws), pack_T(f32("k_w")),
         pack_T(f32("v_w")), owpk], axis=1).astype(bf)
    assert wpack.shape == (128, PACK_COLS)

    vecs = np.stack([shift1.reshape(CC, 128), shiftF.reshape(CC, 128),
                     qbs.reshape(CC, 128)]).reshape(3 * CC, 128).T
    shared = {"wpack": np.ascontiguousarray(wpack),
              "vecs": np.ascontiguousarray(vecs.astype(np.float32))}
    in_maps = []
    for core in range(N_CORES):
        xs = x[core * IMGS:(core + 1) * IMGS].reshape(IMGS, CC, 128, N)
        in_maps.append({"x_sh": np.ascontiguousarray(xs), **shared})
    return in_maps


_NC_CACHE = {}


def _get_nc():
    if "nc" not in _NC_CACHE:
        _NC_CACHE["nc"] = build_nc()
    return _NC_CACHE["nc"]


def kernel(**inputs) -> np.ndarray:
    nc = _get_nc()
    in_maps = _prep_inputs(inputs)
    res = run_bass_kernel_spmd(nc, in_maps, core_ids=list(range(N_CORES)))
    outs = [res.results[i]["out_sh"].reshape(IMGS, C, H, W) for i in range(N_CORES)]
    return np.concatenate(outs, axis=0)
